# revision 11
# baseline (speedup 1.0000x reference)
"""Trainium2 Bass kernel: 2-layer MLP whose "linear" layers are
    mean_i(x[:, :, None] * W[None] + b)  ==  x @ W / D_in + mean_i(b)
so the real work is streaming the huge per-sample bias tensors
b1 (B,2048,1024) / b2 (B,1024,1000) from HBM and reducing over axis 1.

Strategy (data parallel over batch, perfectly balanced at 12.5 samples
per core: 12 full samples each, plus samples 96-99 split in half by
reduction rows across core pairs; the host combines the two half-sums
and does the 4 residual rows' tiny layer math in numpy):
  - All reductions run on the TensorEngine: sum_i b[s, i, :] is a
    matmul with a one-hot column of an identity "mask" as the
    stationary operand (column 12 = the residual half-sample), which
    accumulates directly into the same PSUM tile as the x@W matmul.
    float32r dtype gives full-rate (1 col/cycle) fp32 streaming.
  - b1/b2 stream through SBUF as 4MB DMAs of (128, 8, D) with row
    blocks laid out so every partition reads 32KB contiguous.
  - relu( (x@W1 + sum b1) / 2048 ) on ScalarE, h transposed via PE,
    then (h@W2 + sum b2) / 1024.
Roofline: ~168.7MB/core of HBM traffic at ~358GB/s => ~471us.
"""

import sys

if "/opt/trn_rl_repo" not in sys.path:
    sys.path.insert(0, "/opt/trn_rl_repo")

import numpy as np

import concourse.bass as bass
import concourse.mybir as mybir
import concourse.tile as tile
from concourse import bacc
from concourse.bass_utils import run_bass_kernel_spmd
from concourse.masks import make_identity

BF = 12  # full samples per core
M = BF + 1  # matmul M width: 12 full samples + 1 residual partial-sum row
BTOT = 100
DIN, DH, DOUT = 2048, 1024, 1000
NCORES = 8

F32 = mybir.dt.float32
F32R = mybir.dt.float32r
AF = mybir.ActivationFunctionType


def _build_nc():
    nc = bacc.Bacc(
        "TRN2",
        target_bir_lowering=False,
        debug=False,
        enable_asserts=False,
        num_devices=NCORES,
    )
    x_d = nc.dram_tensor("x", [BF, DIN], F32, kind="ExternalInput").ap()
    w1_d = nc.dram_tensor("W1", [DIN, DH], F32, kind="ExternalInput").ap()
    b1_d = nc.dram_tensor("b1", [BF, DIN, DH], F32R, kind="ExternalInput").ap()
    b1h_d = nc.dram_tensor("b1h", [DIN // 2, DH], F32R, kind="ExternalInput").ap()
    w2_d = nc.dram_tensor("W2", [DH, DOUT], F32, kind="ExternalInput").ap()
    b2_d = nc.dram_tensor("b2", [BF, DH, DOUT], F32R, kind="ExternalInput").ap()
    b2h_d = nc.dram_tensor("b2h", [DH // 2, DOUT], F32R, kind="ExternalInput").ap()
    # out rows 0..11 = full samples; row 12 = (sum_j b2h)/DH partial
    out_d = nc.dram_tensor("out", [M, DOUT], F32, kind="ExternalOutput").ap()
    p1_d = nc.dram_tensor("p1", [1, DH], F32, kind="ExternalOutput").ap()

    with tile.TileContext(nc) as tc:
        with (
            tc.tile_pool(name="const", bufs=1) as constp,
            tc.tile_pool(name="stream", bufs=3) as streamp,
            tc.tile_pool(name="resid", bufs=1) as residp,
            tc.tile_pool(name="wpool", bufs=2) as wpool,
            tc.tile_pool(name="psum", bufs=1, space="PSUM") as psump,
            tc.tile_pool(name="psumt", bufs=2, space="PSUM") as psumtp,
        ):
            # W1 chunk 0 + x first so the HBM stream starts immediately
            w1ts = []
            for r in range(4):
                w1t = wpool.tile([128, 4, DH], F32, tag="w")
                nc.sync.dma_start(
                    out=w1t,
                    in_=w1_d[r * 512 : (r + 1) * 512, :].rearrange(
                        "(p c) m -> p c m", p=128
                    ),
                )
                w1ts.append(w1t)
                if r == 0:
                    x_sb = constp.tile([BF, DIN], F32)
                    nc.sync.dma_start(out=x_sb, in_=x_d)

            # ---- constants ----
            ident = constp.tile([BF, BF], F32)
            make_identity(nc, ident)
            # mask[:, s, m] = 1.0 iff s == m : column s of the "ones" weights
            # (col 12 doubles as the residual-half accumulator row selector;
            #  built as f32 — walrus rejects f32r memsets — used bitcast f32r)
            mask_f = constp.tile([128, M, M], F32)
            nc.vector.memset(mask_f, 0.0)
            for s in range(M):
                nc.vector.memset(mask_f[:, s, s : s + 1], 1.0)

            # xT[:, k, b] = x[b, col(k, p)] matching W1's (p c) row layout
            xT = constp.tile([128, 16, BF], F32)
            for r in range(4):
                xs = x_sb[:, r * 512 : (r + 1) * 512].rearrange(
                    "b (p c) -> b c p", p=128, c=4
                )
                for c in range(4):
                    pt = psumtp.tile([128, BF], F32, tag="tp")
                    nc.tensor.transpose(pt, xs[:, c, :], ident)
                    nc.any.tensor_copy(out=xT[:, r * 4 + c, :], in_=pt)

            # open psum_o's accumulation groups immediately with zero matmuls
            # so the b2 stream (and the mid-stream b2 residual) never waits on
            # the relu/transpose chain
            psum_o = psump.tile([M, DOUT], F32)
            nhalves = ((0, 512), (512, DOUT - 512))
            zcol = constp.tile([128, M], F32)
            nc.vector.memset(zcol, 0.0)
            zrhs = constp.tile([128, 512], F32)
            nc.vector.memset(zrhs, 0.0)
            for off, n in nhalves:
                nc.tensor.matmul(
                    psum_o[:, off : off + n],
                    zcol.bitcast(F32R),
                    zrhs[:, 0:n].bitcast(F32R),
                    start=True,
                    stop=False,
                )

            # ---- layer 1: psum_h = x @ W1 + sum_i b1 ----
            psum_h = psump.tile([BF, DH], F32)
            for r in range(4):
                for c in range(4):
                    k = r * 4 + c
                    for h in range(2):
                        nc.tensor.matmul(
                            psum_h[:, h * 512 : (h + 1) * 512],
                            xT[:, k, :],
                            w1ts[r][:, c, h * 512 : (h + 1) * 512],
                            start=(k == 0),
                            stop=False,
                        )

            # residual b1 half-sample -> its own M=1 PSUM tile (engines can
            # only address PSUM at 32-aligned base partitions, so it can't
            # share psum_h); lhsT = all-ones column mask_f[:, 12, 12:13]
            psum_p1 = psump.tile([1, DH], F32)
            th1 = residp.tile([128, 8, DH], F32R, tag="resid")
            nc.sync.dma_start(out=th1, in_=b1h_d.rearrange("(p c) m -> p c m", p=128))
            for c in range(8):
                for h in range(2):
                    nc.tensor.matmul(
                        psum_p1[:, h * 512 : (h + 1) * 512],
                        mask_f[:, BF, BF : BF + 1].bitcast(F32R),
                        th1[:, c, h * 512 : (h + 1) * 512],
                        start=(c == 0),
                        stop=(c == 7),
                    )

            for b in range(BF):  # full-sample b1 stream: 2 x 4MB DMAs each
                for r in range(2):
                    last_dma = b == BF - 1 and r == 1
                    # split the final DMA so its matmuls pipeline with arrival
                    parts = ((0, 4), (4, 4)) if last_dma else ((0, 8),)
                    src = b1_d[b, r * 1024 : (r + 1) * 1024, :].rearrange(
                        "(p c) m -> p c m", p=128
                    )
                    for c0, cn in parts:
                        t1 = streamp.tile([128, cn, DH], F32R, tag="stream")
                        nc.sync.dma_start(out=t1, in_=src[:, c0 : c0 + cn, :])
                        for ci in range(cn):
                            for h in range(2):
                                nc.tensor.matmul(
                                    psum_h[:, h * 512 : (h + 1) * 512],
                                    mask_f[:, b, 0:BF].bitcast(F32R),
                                    t1[:, ci, h * 512 : (h + 1) * 512],
                                    start=False,
                                    stop=(last_dma and c0 + ci == 7),
                                )
                if b == 5:
                    # mid-stream: residual b2 half-sample -> psum_o row 12
                    th2 = residp.tile([128, 4, DOUT], F32R, tag="resid")
                    nc.sync.dma_start(
                        out=th2, in_=b2h_d.rearrange("(p c) m -> p c m", p=128)
                    )
                    for c in range(4):
                        for off, n in nhalves:
                            nc.tensor.matmul(
                                psum_o[:, off : off + n],
                                mask_f[:, BF, :].bitcast(F32R),
                                th2[:, c, off : off + n],
                                start=False,
                                stop=False,
                            )

            # W2 queued right behind the b1 bytes
            w2ts = []
            for r in range(2):
                w2t = wpool.tile([128, 4, DOUT], F32, tag="w")
                nc.sync.dma_start(
                    out=w2t,
                    in_=w2_d[r * 512 : (r + 1) * 512, :].rearrange(
                        "(p c) m -> p c m", p=128
                    ),
                )
                w2ts.append(w2t)

            # ---- h = relu(psum_h / DIN); p1 partial out; hT for layer 2 ----
            h_sb = constp.tile([BF, DH], F32)
            nc.scalar.activation(out=h_sb, in_=psum_h, func=AF.Relu, scale=1.0 / DIN)
            p1_sb = constp.tile([1, DH], F32)
            nc.scalar.activation(out=p1_sb, in_=psum_p1, func=AF.Copy, scale=1.0 / DIN)
            nc.sync.dma_start(out=p1_d, in_=p1_sb)

            hT = constp.tile([128, 8, M], F32)
            nc.vector.memset(hT, 0.0)
            for r in range(2):
                hs = h_sb[:, r * 512 : (r + 1) * 512].rearrange(
                    "b (p c) -> b c p", p=128, c=4
                )
                for c in range(4):
                    pt = psumtp.tile([128, BF], F32, tag="tp")
                    nc.tensor.transpose(pt, hs[:, c, :], ident)
                    nc.any.tensor_copy(out=hT[:, r * 4 + c, 0:BF], in_=pt)

            # ---- layer 2: psum_o += h @ W2 + sum_j b2 ----
            for r in range(2):
                for c in range(4):
                    j = r * 4 + c
                    for off, n in nhalves:
                        nc.tensor.matmul(
                            psum_o[:, off : off + n],
                            hT[:, j, :],
                            w2ts[r][:, c, off : off + n],
                            start=False,
                            stop=False,
                        )

            for b in range(BF):  # full-sample b2 stream: 1 x 4MB DMA each
                last_dma = b == BF - 1
                parts = ((0, 4), (4, 4)) if last_dma else ((0, 8),)
                src = b2_d[b].rearrange("(p c) m -> p c m", p=128)
                for c0, cn in parts:
                    t2 = streamp.tile([128, cn, DOUT], F32R, tag="stream")
                    nc.sync.dma_start(out=t2, in_=src[:, c0 : c0 + cn, :])
                    for ci in range(cn):
                        for off, n in nhalves:
                            nc.tensor.matmul(
                                psum_o[:, off : off + n],
                                mask_f[:, b, :].bitcast(F32R),
                                t2[:, ci, off : off + n],
                                start=False,
                                stop=(last_dma and c0 + ci == 7),
                            )

            # ---- out = psum_o / DH (row 12 = b2 partial mean) ----
            out_sb = constp.tile([M, DOUT], F32)
            nc.scalar.activation(out=out_sb, in_=psum_o, func=AF.Copy, scale=1.0 / DH)
            nc.sync.dma_start(out=out_d, in_=out_sb)

    nc.compile()
    return nc


_CACHE: dict = {}


def _get_nc():
    if "nc" not in _CACHE:
        _CACHE["nc"] = _build_nc()
    return _CACHE["nc"]


def _make_in_maps(x, W1, b1, W2, b2):
    x = np.ascontiguousarray(np.asarray(x, dtype=np.float32))
    W1 = np.ascontiguousarray(np.asarray(W1, dtype=np.float32))
    b1 = np.asarray(b1, dtype=np.float32)
    W2 = np.ascontiguousarray(np.asarray(W2, dtype=np.float32))
    b2 = np.asarray(b2, dtype=np.float32)
    maps = []
    for c in range(NCORES):
        s = BF * c
        rs = 8 * BF + c // 2  # residual sample id (96..99)
        hh = c % 2  # which half of its reduction rows this core sums
        maps.append(
            {
                "x": x[s : s + BF],
                "W1": W1,
                "b1": b1[s : s + BF],
                "b1h": b1[rs, hh * (DIN // 2) : (hh + 1) * (DIN // 2), :],
                "W2": W2,
                "b2": b2[s : s + BF],
                "b2h": b2[rs, hh * (DH // 2) : (hh + 1) * (DH // 2), :],
            }
        )
    return maps


def _axon_reset():
    try:
        import ctypes

        lib = ctypes.CDLL("/opt/axon/libaxon_pjrt.so")
        lib.axon_reset.restype = ctypes.c_int64
        lib.axon_reset()
    except Exception:
        pass


def _run(in_maps, **kw):
    try:
        return run_bass_kernel_spmd(_get_nc(), in_maps, list(range(NCORES)), **kw)
    except Exception:
        # one retry after a device reset (NRT_EXEC_UNIT_UNRECOVERABLE etc.)
        _axon_reset()
        return run_bass_kernel_spmd(_get_nc(), in_maps, list(range(NCORES)), **kw)


def _assemble(results, x, W1, W2):
    out = np.empty((BTOT, DOUT), np.float32)
    for c in range(NCORES):
        out[BF * c : BF * (c + 1)] = results[c]["out"][0:BF]
    for k in range(4):  # residual samples: combine the two half-sums
        s = 8 * BF + k
        p1 = results[2 * k]["p1"][0] + results[2 * k + 1]["p1"][0]  # mean_i b1[s]
        p2 = results[2 * k]["out"][BF] + results[2 * k + 1]["out"][BF]  # mean_j b2[s]
        h = np.maximum(x[s] @ W1 / np.float32(DIN) + p1, 0.0)
        out[s] = h @ W2 / np.float32(DH) + p2
    return out


def kernel(x, W1, b1, W2, b2):
    x = np.asarray(x, dtype=np.float32)
    W1 = np.asarray(W1, dtype=np.float32)
    W2 = np.asarray(W2, dtype=np.float32)
    res = _run(_make_in_maps(x, W1, b1, W2, b2)).results
    return _assemble(res, x, W1, W2)
